# revision 1
# baseline (speedup 1.0000x reference)
"""Trainium2 Bass kernel for nn_MixedAttention.

Full inputs in, full output out. Sharding: 8 cores = 2 (batch) x 4 (head
pairs). Each core computes 2 global + 2 local heads for one batch element.

Key algebraic rewrite for the local branch:
    lscores = (lq@lk1^T)@(lk1@lk2^T) = lq @ (lk1^T@lk1) @ lk2^T
with M = lk1^T@lk1 a [64,64] matrix -- turns a 2048^3 matmul chain into
two small matmuls plus one S x S matmul (30x less PE work).

Precision/dtype strategy: fp32 matmuls run at ~2 cycles/column on the PE,
float32r (TF32-like, ~13-bit mantissa) at ~1. The local-branch scores are
large (|raw| up to ~2000) and feed exp(), so any input rounding there is
amplified exponentially -> the local score chain (hidden^T, score-side
projections, pass-2 score matmul) stays fp32. Everything whose error is
not exp-amplified runs f32r: global q/k (scores |s|<~5), all value paths,
the probs @ v context matmuls, and the local pass-1 max (only needs ~1 ulp
of exp range).

Layout: scores are computed transposed st[j, i] = K_eff @ Q_eff^T so the
context matmul needs no transposed probs (lhsT = v_nat, rhs = e). v gets
an extra ones column so the softmax denominator falls out of the context
matmul for free. Global heads skip max subtraction entirely (mask folded
into the Exp bias); local heads get an exact row max from a separate
f32r pass in the untransposed orientation (free-dim reduce_max), and the
-max correction rides an extra contraction row (K=65) in pass 2.
"""

import math
import os
import sys

import numpy as np

sys.path.insert(0, "/opt/trn_rl_repo")

B, S, HID, HEAD = 2, 2048, 1024, 64
SC = S // 128  # 16 s-chunks of 128
HC = HID // 128  # 8 hidden chunks
N_CORES = 8
SCALE = 1.0 / math.sqrt(HEAD)

W_NAMES = ["wq", "wk", "wv", "wlq", "wlk1", "wlk2", "wlv"]
F32R_PROJ = {"wq", "wk", "wv"}  # projections written as f32r at the source

_CACHE = {}
LAST_RESULTS = None  # stash of BassKernelResults for test.py profiling


def _build():
    import concourse.mybir as mybir
    import concourse.tile as tile
    from concourse import bacc
    from concourse.masks import make_identity

    f32 = mybir.dt.float32
    f32r = mybir.dt.float32r
    AF = mybir.ActivationFunctionType
    ALU = mybir.AluOpType
    AX = mybir.AxisListType

    nc = bacc.Bacc("TRN2", target_bir_lowering=False, debug=False,
                   enable_asserts=False)

    hid_d = nc.dram_tensor("hid", (HID, S), f32, kind="ExternalInput").ap()
    mask_d = nc.dram_tensor("mask", (S,), f32, kind="ExternalInput").ap()
    w_d = {n: nc.dram_tensor(n, (HID, 128), f32, kind="ExternalInput").ap()
           for n in W_NAMES}
    b_d = {n: nc.dram_tensor("b" + n[1:], (128,), f32,
                             kind="ExternalInput").ap() for n in W_NAMES}
    out_d = nc.dram_tensor("out", (S, 256), f32, kind="ExternalOutput").ap()

    with tile.TileContext(nc) as tc:
        with (
            tc.tile_pool(name="const", bufs=1) as constp,
            tc.tile_pool(name="persist", bufs=1) as pp,
            tc.tile_pool(name="wp_g", bufs=1) as wp_g,
            tc.tile_pool(name="epool", bufs=8) as ep,
            tc.tile_pool(name="opool", bufs=1) as op_,
            tc.tile_pool(name="ps_tr", bufs=2, space="PSUM") as ps_tr,
            tc.tile_pool(name="ps_mm", bufs=4, space="PSUM") as ps_mm,
            tc.tile_pool(name="ps_ctx", bufs=2, space="PSUM") as ps_ctx,
            tc.tile_pool(name="dramp", bufs=2, space="DRAM") as dramp,
        ):
            ident = constp.tile([128, 128], f32, name="ident")
            make_identity(nc, ident)
            identr = constp.tile([128, 128], f32r, name="identr")
            nc.vector.tensor_copy(identr, ident)
            ones_sb = constp.tile([128, SC], f32, name="ones_sb")
            nc.vector.memset(ones_sb, 1.0)
            mask_sb = constp.tile([128, SC], f32, name="mask_sb")
            nc.gpsimd.dma_start(mask_sb,
                                mask_d.rearrange("(c p) -> p c", p=128))
            bias_sb = {}
            for n in W_NAMES:
                t = constp.tile([128, 1], f32, name=f"b_{n}")
                nc.gpsimd.dma_start(t, b_d[n][:, None])
                bias_sb[n] = t

            projT = {n: pp.tile([128, S], f32, name=f"projT_{n}")
                     for n in W_NAMES if n not in F32R_PROJ}

            out_sb = op_.tile([128, SC, 256], f32, name="out_sb")

            # ---------- emission helpers ----------

            def emit_wdma(n, iop):
                wsb = iop.tile([128, HC, 128], f32, tag="w", name=f"w_{n}")
                nc.sync.dma_start(
                    wsb, w_d[n].rearrange("(c p) m -> p c m", p=128))
                return wsb

            def emit_proj_half(n, wsb, hidT, half):
                accs = [ps_mm.tile([128, 512], f32, tag="mm",
                                   name=f"acc{i}") for i in range(2)]
                for hc in range(HC):
                    for ic in range(2):
                        icg = half * 2 + ic
                        nc.tensor.matmul(
                            accs[ic], lhsT=wsb[:, hc],
                            rhs=hidT[:, hc, icg * 512:(icg + 1) * 512],
                            start=(hc == 0), stop=(hc == HC - 1))
                for ic in range(2):
                    icg = half * 2 + ic
                    nc.vector.tensor_scalar_add(
                        projT[n][:, icg * 512:(icg + 1) * 512],
                        accs[ic], bias_sb[n])

            def emit_proj(n, hidT, iop):
                wsb = emit_wdma(n, iop)
                for half in range(2):
                    emit_proj_half(n, wsb, hidT, half)

            def build_vaug(vT, vdt):
                # v natural [s, d] + ones column -> [128, SC, 65] f32r
                idm = identr if vdt == f32r else ident
                base = vT.base_partition()
                idsl = slice(base, base + 64)
                vaug = wp_g.tile([128, SC, 65], f32r, tag="vaug",
                                 name="vaug", bufs=2)
                nc.vector.tensor_copy(vaug[:, :, 64], ones_sb)
                for t in range(SC):
                    pt = ps_tr.tile([128, 128], vdt, tag="tr")
                    nc.tensor.transpose(
                        pt[:, :64], vT[:, t * 128:(t + 1) * 128],
                        idm[idsl, idsl])
                    nc.any.tensor_copy(vaug[:, t, :64], pt[:, :64])
                return vaug

            def attention_ic(head, kT, qT, vaug, is_local, ic):
                # main pass: st -> exp -> ctx (+sums via the ones column),
                # then transpose back and divide by the sums
                csl = slice(head * 64, (head + 1) * 64)
                if True:
                    isl = slice(ic * 512, (ic + 1) * 512)
                    ctx = ps_ctx.tile([65, 512], f32, tag="ctx", name="ctx")

                    def ctx_group(es):
                        for jc, e in es:
                            nc.tensor.matmul(ctx, lhsT=vaug[:, jc], rhs=e,
                                             start=(jc == 0),
                                             stop=(jc == SC - 1))

                    prev = None
                    for jg in range(4):
                        es = []
                        for jj in range(4):
                            jc = jg * 4 + jj
                            jsl = slice(jc * 128, (jc + 1) * 128)
                            st = ps_mm.tile([128, 512], f32, tag="mm",
                                            name="st")
                            nc.tensor.matmul(st, lhsT=kT[:, jsl],
                                             rhs=qT[:, isl],
                                             start=True, stop=True)
                            e = ep.tile([128, 512], f32r, tag="e", name="e")
                            bias = 0.0 if is_local else mask_sb[:, jc:jc + 1]
                            nc.scalar.activation(e, st, AF.Exp, bias=bias,
                                                 scale=SCALE)
                            es.append((jc, e))
                        if prev is not None:
                            ctx_group(prev)
                        prev = es
                    ctx_group(prev)
                    ctx_sbc = wp_g.tile([65, 512], f32, tag="ctx_sbc",
                                        name="ctx_sbc")
                    nc.any.tensor_copy(ctx_sbc, ctx)
                    for tt in range(4):
                        t = ic * 4 + tt
                        pt = ps_tr.tile([128, 128], f32, tag="tr")
                        nc.tensor.transpose(
                            pt[:, :65], ctx_sbc[:, tt * 128:(tt + 1) * 128],
                            ident[:65, :65])
                        rec = wp_g.tile([128, 1], f32, tag="rec", name="rec")
                        nc.vector.reciprocal(rec, pt[:, 64:65])
                        nc.vector.tensor_scalar_mul(
                            out_sb[:, t, csl], pt[:, :64], rec)
                    nc.sync.dma_start(
                        out_d.rearrange("(t p) c -> p t c", p=128)[
                            :, ic * 4:(ic + 1) * 4, csl],
                        out_sb[:, ic * 4:(ic + 1) * 4, csl])

            def local_prep(head, wp):
                hh = head % 2
                rs = slice(hh * 64, (hh + 1) * 64)
                if hh == 0:
                    lqT = projT["wlq"][rs]
                    lk1T = projT["wlk1"][rs]
                else:
                    lqT = wp.tile([64, S], f32, tag="s0l", name="s0l")
                    nc.scalar.copy(lqT, projT["wlq"][rs])
                    lk1T = wp.tile([64, S], f32, tag="s1l", name="s1l")
                    nc.scalar.copy(lk1T, projT["wlk1"][rs])

                # lk1 natural [s, d] via transposes
                lk1nat = wp.tile([128, SC, 64], f32, tag="lk1nat",
                                 name="lk1nat", bufs=2)
                for t in range(SC):
                    pt = ps_tr.tile([128, 128], f32, tag="tr")
                    nc.tensor.transpose(
                        pt[:, :64], lk1T[:, t * 128:(t + 1) * 128],
                        ident[:64, :64])
                    nc.any.tensor_copy(lk1nat[:, t], pt[:, :64])
                # M = lk1^T @ lk1 [64, 64] (symmetric)
                mps = ps_mm.tile([128, 512], f32, tag="mm", name="mps")
                for t in range(SC):
                    nc.tensor.matmul(mps[:64, :64], lhsT=lk1nat[:, t],
                                     rhs=lk1nat[:, t],
                                     start=(t == 0), stop=(t == SC - 1))
                m_sb = wp.tile([64, 64], f32, tag="m_sb", name="m_sb",
                               bufs=2)
                nc.any.tensor_copy(m_sb, mps[:64, :64])
                # qaug rows 0:64 = (lq @ M)^T = M @ lq^T (M symmetric);
                # row 64 filled later with -max
                qaug = wp.tile([65, S], f32, tag="qaug", name="qaug",
                               bufs=2)
                for ic in range(4):
                    mm = ps_mm.tile([128, 512], f32, tag="mm", name="mm")
                    nc.tensor.matmul(mm[:64], lhsT=m_sb,
                                     rhs=lqT[:, ic * 512:(ic + 1) * 512],
                                     start=True, stop=True)
                    nc.any.tensor_copy(qaug[:64, ic * 512:(ic + 1) * 512],
                                       mm[:64])
                # k2aug: rows 0:64 = lk2^T, row 64 = ones
                k2aug = wp.tile([65, S], f32, tag="k2aug", name="k2aug",
                                bufs=2)
                nc.scalar.copy(k2aug[:64, :], projT["wlk2"][rs])
                nc.vector.memset(k2aug[64:65, :], 1.0)
                vaug = build_vaug(projT["wlv"][rs], f32)

                # f32r shadows for pass 1 (max only needs ~1 absolute)
                qaug_r = wp.tile([64, S], f32r, tag="qaug_r", name="qaug_r",
                                 bufs=2)
                nc.scalar.copy(qaug_r, qaug[:64])
                k2aug_r = wp.tile([64, S], f32r, tag="k2aug_r",
                                  name="k2aug_r", bufs=2)
                nc.scalar.copy(k2aug_r, k2aug[:64])
                return dict(qaug=qaug, k2aug=k2aug, vaug=vaug,
                            qaug_r=qaug_r, k2aug_r=k2aug_r)

            def local_pass1(head, hs, wp):
                # pass 1: untransposed s[i, j] blocks; row max via free-dim
                # reduce (independent ops; no serial DVE chain)
                qaug_r, k2aug_r = hs["qaug_r"], hs["k2aug_r"]
                maxneg = wp.tile([128, SC], f32, tag="maxneg", name="maxneg",
                                 bufs=2)
                for t in range(SC):
                    pmax = wp.tile([128, 4], f32, tag="pmax", name="pmax",
                                   bufs=2)
                    for j4 in range(4):
                        st = ps_mm.tile([128, 512], f32, tag="mm", name="st1")
                        nc.tensor.matmul(
                            st, lhsT=qaug_r[:, t * 128:(t + 1) * 128],
                            rhs=k2aug_r[:, j4 * 512:(j4 + 1) * 512],
                            start=True, stop=True)
                        nc.vector.tensor_reduce(pmax[:, j4:j4 + 1], st,
                                                axis=AX.X, op=ALU.max)
                    nc.vector.tensor_reduce(maxneg[:, t:t + 1], pmax,
                                            axis=AX.X, op=ALU.max,
                                            negate=True)
                mscr = dramp.tile([S], f32, tag="mscr", name="mscr")
                nc.sync.dma_start(
                    mscr.rearrange("(t p) -> p t", p=128), maxneg)
                nc.sync.dma_start(hs["qaug"][64:65, :], mscr[None, :])

                        # ---------- phase A: hidden^T, projections, global heads ----
            with (
                tc.tile_pool(name="pp_g", bufs=1) as pp_g,
                tc.tile_pool(name="hidT", bufs=1) as hp,
                tc.tile_pool(name="io", bufs=2) as iop,
            ):
                for n in F32R_PROJ:
                    projT[n] = pp_g.tile([128, S], f32r, name=f"projT_{n}")
                hidT = hp.tile([128, HC, S], f32, name="hidT")
                hid_r = hid_d.rearrange("(c p) s -> p c s", p=128)
                # qkv weights first on the gpsimd queue so the first
                # projection matmuls start as soon as hidT chunk 0 lands
                wsb_g = {}
                for n in ["wq", "wk", "wv"]:
                    wsb_g[n] = iop.tile([128, HC, 128], f32, tag="wg",
                                        name=f"w_{n}")
                    nc.gpsimd.dma_start(
                        wsb_g[n], w_d[n].rearrange("(c p) m -> p c m", p=128))
                for hc in range(HC):
                    eng = nc.sync if hc % 2 == 0 else nc.gpsimd
                    eng.dma_start(hidT[:, hc], hid_r[:, hc])
                for n in ["wq", "wk", "wv"]:
                    for half in range(2):
                        emit_proj_half(n, wsb_g[n], hidT, half)
                gvaug = {}
                for hh in range(2):
                    rs = slice(hh * 64, (hh + 1) * 64)
                    gvaug[hh] = build_vaug(projT["wv"][rs], f32r)
                # interleave: global-head attention units between local
                # projection halves so the in-order PE queue always has
                # independent matmuls (keeps HAM warm)
                lp = [(n, half) for n in ["wlq", "wlk1", "wlk2", "wlv"]
                      for half in range(2)]
                wsbs = {}
                for i, (hh, ic) in enumerate(
                        [(h, c) for h in range(2) for c in range(4)]):
                    rs = slice(hh * 64, (hh + 1) * 64)
                    attention_ic(hh, projT["wk"][rs], projT["wq"][rs],
                                 gvaug[hh], False, ic)
                    n, half = lp[i]
                    if half == 0:
                        wsbs[n] = emit_wdma(n, iop)
                    emit_proj_half(n, wsbs[n], hidT, half)

            # ---------- phase B: local heads (stage-interleaved so
            # the PE never idles long enough to go HAM-cold) ----------
            with tc.tile_pool(name="wp_l", bufs=1) as wp_l:
                st2 = local_prep(2, wp_l)
                st3 = local_prep(3, wp_l)
                local_pass1(2, st2, wp_l)
                local_pass1(3, st3, wp_l)
                for ic in range(4):
                    attention_ic(2, st2["k2aug"], st2["qaug"], st2["vaug"],
                                 True, ic)
                for ic in range(4):
                    attention_ic(3, st3["k2aug"], st3["qaug"], st3["vaug"],
                                 True, ic)

    nc.compile()
    return nc


def _patch_ldw_opt():
    # walrus ships with the LDWEIGHTS optimizer disabled; fp32 matmuls
    # pay a bundled weight reload per matmul, so try enabling the
    # optimizer (verified against the reference output by the caller).
    from concourse import bass_utils
    if getattr(bass_utils, "_ldw_patched", False):
        return
    orig = bass_utils.bir_verify_and_optimise

    def patched(*a, **k):
        import subprocess
        orig_run = bass_utils.run_command

        def run2(cmd, **kw):
            cmd = [c.replace("--enable-ldw-opt=false",
                             "--enable-ldw-opt=true") for c in cmd]
            return orig_run(cmd, **kw)

        bass_utils.run_command = run2
        try:
            return orig(*a, **k)
        finally:
            bass_utils.run_command = orig_run

    bass_utils.bir_verify_and_optimise = patched
    bass_utils._ldw_patched = True


def kernel(**inputs):
    from concourse import bass_utils

    if os.environ.get("LDW_OPT", "0") == "1":
        _patch_ldw_opt()

    global LAST_RESULTS
    if "nc" not in _CACHE:
        _CACHE["nc"] = _build()
    nc = _CACHE["nc"]

    inputs = dict(inputs)
    inputs["wlv"] = np.asarray(inputs["wlv1"]) + np.asarray(inputs["wlv2"])
    inputs["blv"] = np.asarray(inputs["blv1"]) + np.asarray(inputs["blv2"])
    hs = np.ascontiguousarray(np.asarray(inputs["hidden_states"], np.float32))
    am = np.ascontiguousarray(np.asarray(inputs["attention_mask"], np.float32))
    in_maps = []
    for c in range(N_CORES):
        b, g = c // 4, c % 4
        csl = slice(128 * g, 128 * (g + 1))
        m = {"hid": np.ascontiguousarray(hs[b].T), "mask": am[b, 0, 0]}
        for n in W_NAMES:
            m[n] = np.ascontiguousarray(
                np.asarray(inputs[n], np.float32)[:, csl])
            m["b" + n[1:]] = np.ascontiguousarray(
                np.asarray(inputs["b" + n[1:]], np.float32)[csl])
        in_maps.append(m)

    res = bass_utils.run_bass_kernel_spmd(
        nc, in_maps, list(range(N_CORES)),
        tmpdir=os.environ.get("BASS_TMPDIR"))
    LAST_RESULTS = res

    out = np.zeros((B, S, HID), np.float32)
    for c in range(N_CORES):
        b, g = c // 4, c % 4
        o = res.results[c]["out"]
        out[b, :, 128 * g:128 * (g + 1)] = o[:, :128]
        out[b, :, 512 + 128 * g:512 + 128 * (g + 1)] = o[:, 128:]
    return out



# revision 14
# speedup vs baseline: 1.2743x; 1.2743x over previous
"""Trainium2 Bass kernel for nn_MixedAttention.

Full inputs in, full output out. Sharding: 8 cores = 2 (batch) x 4 (head
pairs). Each core computes 2 global + 2 local heads for one batch element.

Key algebraic rewrite for the local branch:
    lscores = (lq@lk1^T)@(lk1@lk2^T) = lq @ (lk1^T@lk1) @ lk2^T
with M = lk1^T@lk1 a [64,64] matrix -- turns a 2048^3 matmul chain into
two small matmuls plus one S x S matmul (30x less PE work).

Precision/dtype strategy (PE cost: fp32=4 cyc/row, f32r/bf16=1, fp8
DoubleRow=0.5). The local score chain needs ~1e-6 relative accuracy
(scores reach |s|~5000 in exponent units and softmax is near-one-hot, so
score error d flips argmaxes on ~P(top2 gap < d) rows). Instead of fp32
matmuls we use exact split pairs: x = hi + lo with hi truncated to 10
explicit mantissa bits (exactly representable in the PE's ~13-bit f32r,
so it passes through unrounded) and lo the f32 residual. Then
  x@w = hi@whi + hi@wlo + lo@whi (+ dropped lo@wlo ~ 2^-20)
runs at 3 cyc/row (projections) and the pass-2 score matmul runs at
2 cyc/row by stacking the cross terms into one K=128 matmul:
  st = [k2hi;ones]^T @ [qhi;-max]  +  [k2lo;k2hi]^T @ [qhi;qlo].
Everything not exp-amplified runs f32r single (global q/k/v, local v,
ctx matmuls) or fp8e4 DoubleRow (global ctx: probs are near-uniform,
N_eff~1600, so fp8 quantization of e and v averages out).
CPU-simulated end-to-end rel err of this scheme: ~1.2e-3 (gate 2e-2).

Layout: scores are computed transposed st[j, i] = K_eff @ Q_eff^T so the
context matmul needs no transposed probs (lhsT = v_nat, rhs = e). v gets
an extra ones column so the softmax denominator falls out of the context
matmul for free. Global heads skip max subtraction entirely (mask folded
into the Exp bias); local heads get an exact row max from a hi-only f32r
pass in the untransposed orientation (free-dim reduce_max), and the
-max correction rides the K=65 contraction row of pass 2.
"""

import math
import os
import sys

import numpy as np

sys.path.insert(0, "/opt/trn_rl_repo")

B, S, HID, HEAD = 2, 2048, 1024, 64
SC = S // 128  # 16 s-chunks of 128
HC = HID // 128  # 8 hidden chunks
N_CORES = 8
SCALE = 1.0 / math.sqrt(HEAD)

W_GLOBAL = ["wq", "wk", "wv", "wlv"]  # f32r hi-only projections
W_LOCAL = ["wlq", "wlk1", "wlk2"]  # split-pair projections (score chain)
W_NAMES = W_GLOBAL + W_LOCAL

_CACHE = {}
LAST_RESULTS = None  # stash of BassKernelResults for test.py profiling


def _trunc10(x):
    # keep 10 explicit mantissa bits -- exactly representable in PE f32r
    b = np.ascontiguousarray(x, np.float32).view(np.int32)
    return (b & ~np.int32(0x1FFF)).view(np.float32)


def _build():
    import concourse.mybir as mybir
    import concourse.tile as tile
    from concourse import bacc
    from concourse.masks import make_identity

    f32 = mybir.dt.float32
    f32r = mybir.dt.float32r
    f8 = mybir.dt.float8e4
    AF = mybir.ActivationFunctionType
    ALU = mybir.AluOpType
    AX = mybir.AxisListType
    DR = mybir.MatmulPerfMode.DoubleRow

    nc = bacc.Bacc("TRN2", target_bir_lowering=False, debug=False,
                   enable_asserts=False)

    hhi_d = nc.dram_tensor("hhi", (HID, S), f32r, kind="ExternalInput").ap()
    hlo_d = nc.dram_tensor("hlo", (HID, S), f32r, kind="ExternalInput").ap()
    mask_d = nc.dram_tensor("mask", (S,), f32, kind="ExternalInput").ap()
    whi_d = {n: nc.dram_tensor("hi_" + n, (HID, 128), f32r,
                               kind="ExternalInput").ap() for n in W_NAMES}
    wlo_d = {n: nc.dram_tensor("lo_" + n, (HID, 128), f32r,
                               kind="ExternalInput").ap() for n in W_LOCAL}
    b_d = {n: nc.dram_tensor("b_" + n, (128,), f32,
                             kind="ExternalInput").ap() for n in W_NAMES}
    out_d = nc.dram_tensor("out", (S, 256), f32, kind="ExternalOutput").ap()

    with tile.TileContext(nc) as tc:
        with (
            tc.tile_pool(name="const", bufs=1) as constp,
            tc.tile_pool(name="persist", bufs=1) as pp,
            tc.tile_pool(name="wp_g", bufs=1) as wp_g,
            tc.tile_pool(name="epool", bufs=4) as ep,
            tc.tile_pool(name="opool", bufs=1) as op_,
            tc.tile_pool(name="ps_tr", bufs=2, space="PSUM") as ps_tr,
            tc.tile_pool(name="ps_mm", bufs=4, space="PSUM") as ps_mm,
            tc.tile_pool(name="ps_ctx", bufs=2, space="PSUM") as ps_ctx,
            tc.tile_pool(name="dramp", bufs=2, space="DRAM") as dramp,
        ):
            ident = constp.tile([128, 128], f32, name="ident")
            make_identity(nc, ident)
            identr = constp.tile([128, 128], f32r, name="identr")
            nc.vector.tensor_copy(identr, ident)
            ones_sb = constp.tile([128, SC], f32, name="ones_sb")
            nc.vector.memset(ones_sb, 1.0)
            mask_sb = constp.tile([128, SC], f32, name="mask_sb")
            nc.gpsimd.dma_start(mask_sb,
                                mask_d.rearrange("(c p) -> p c", p=128))
            bias_sb = {}
            for n in W_NAMES:
                t = constp.tile([128, 1], f32, name=f"b_{n}")
                nc.gpsimd.dma_start(t, b_d[n][:, None])
                bias_sb[n] = t

            # persistent (phase A + B) transposed projections
            projT = {n: pp.tile([128, S], f32, name=f"projT_{n}")
                     for n in W_LOCAL}
            projT["wlv"] = pp.tile([128, S], f32r, name="projT_wlv")

            out_sb = op_.tile([128, SC, 256], f32, name="out_sb")

            # ---------- shared output path ----------

            def out_path(ctx, head, ic):
                csl = slice(head * 64, (head + 1) * 64)
                ctx_sbc = wp_g.tile([65, 512], f32, tag="ctx_sbc",
                                    name="ctx_sbc", bufs=2)
                nc.any.tensor_copy(ctx_sbc, ctx)
                for tt in range(4):
                    t = ic * 4 + tt
                    pt = ps_tr.tile([128, 128], f32, tag="tr")
                    nc.tensor.transpose(
                        pt[:, :65], ctx_sbc[:, tt * 128:(tt + 1) * 128],
                        ident[:65, :65])
                    rec = wp_g.tile([128, 1], f32, tag="rec", name="rec",
                                    bufs=2)
                    nc.vector.reciprocal(rec, pt[:, 64:65])
                    nc.vector.tensor_scalar_mul(
                        out_sb[:, t, csl], pt[:, :64], rec)
                nc.sync.dma_start(
                    out_d.rearrange("(t p) c -> p t c", p=128)[
                        :, ic * 4:(ic + 1) * 4, csl],
                    out_sb[:, ic * 4:(ic + 1) * 4, csl])

            # ---------- global branch ----------

            def global_attention_ic(hh, kT, qT, vaug2, ic):
                isl = slice(ic * 512, (ic + 1) * 512)
                ctx = ps_ctx.tile([65, 512], f32, tag="ctx", name="ctx")

                def ctx_pairs(pairs):
                    for g, e2 in pairs:
                        nc.tensor.matmul(ctx, lhsT=vaug2[:, g, :, :65],
                                         rhs=e2, start=(g == 0),
                                         stop=(g == 7), perf_mode=DR)

                prev = None
                for jg in range(4):
                    pairs = []
                    for jj in range(2):
                        e2 = ep.tile([128, 2, 512], f8, tag="e2", name="e2")
                        for i2 in range(2):
                            jc = jg * 4 + jj * 2 + i2
                            jsl = slice(jc * 128, (jc + 1) * 128)
                            st = ps_mm.tile([128, 512], f32, tag="mm",
                                            name="st")
                            nc.tensor.matmul(st, lhsT=kT[:, jsl],
                                             rhs=qT[:, isl],
                                             start=True, stop=True)
                            nc.scalar.activation(
                                e2[:, i2], st, AF.Exp,
                                bias=mask_sb[:, jc:jc + 1], scale=SCALE)
                        pairs.append((jg * 2 + jj, e2))
                    if prev is not None:
                        ctx_pairs(prev)
                    prev = pairs
                ctx_pairs(prev)
                out_path(ctx, hh, ic)

            def build_vaug2_fp8(vT, gvp):
                # v natural [s, d] + ones col, fp8, paired for DoubleRow:
                # [128, 8, 2, 80] where [:, g, i, d] = v[(2g+i)*128 + p, d]
                base = vT.base_partition()
                idsl = slice(base, base + 64)
                vaug2 = gvp.tile([128, 8, 2, 80], f8, tag="vaug2",
                                 name="vaug2", bufs=2)
                nc.vector.tensor_copy(
                    vaug2[:, :, :, 64],
                    ones_sb.rearrange("p (a b) -> p a b", a=8))
                for t in range(SC):
                    pt = ps_tr.tile([128, 128], f32r, tag="tr")
                    nc.tensor.transpose(
                        pt[:, :64], vT[:, t * 128:(t + 1) * 128],
                        identr[idsl, idsl])
                    nc.any.tensor_copy(vaug2[:, t // 2, t % 2, :64],
                                       pt[:, :64])
                return vaug2

            # ---------- local branch ----------

            def build_vaug_f32r(vT, wp):
                base = vT.base_partition()
                idsl = slice(base, base + 64)
                vaug = wp.tile([128, SC, 65], f32r, tag="vaug",
                               name="vaug", bufs=2)
                nc.vector.tensor_copy(vaug[:, :, 64], ones_sb)
                for t in range(SC):
                    pt = ps_tr.tile([128, 128], f32r, tag="tr")
                    nc.tensor.transpose(
                        pt[:, :64], vT[:, t * 128:(t + 1) * 128],
                        identr[idsl, idsl])
                    nc.any.tensor_copy(vaug[:, t, :64], pt[:, :64])
                return vaug

            def dekker_split(src, hi_dst, hi_dst2, lo_dst, wp):
                # exact split: hi has <=10 explicit mantissa bits (passes
                # through PE f32r rounding unchanged), lo = src - hi
                t1 = wp.tile([64, S], f32, tag="dk1", name="dk1", bufs=1)
                nc.vector.tensor_scalar_mul(t1, src, 8193.0)
                t2 = wp.tile([64, S], f32, tag="dk2", name="dk2", bufs=1)
                nc.vector.tensor_sub(t2, t1, src)
                nc.vector.tensor_sub(hi_dst, t1, t2)
                nc.vector.tensor_sub(lo_dst, src, hi_dst)
                nc.scalar.copy(hi_dst2, hi_dst)

            def local_prep(head, wp):
                hh = head % 2
                rs = slice(hh * 64, (hh + 1) * 64)
                lk1T = projT["wlk1"][rs]
                if hh == 0:
                    lqT = projT["wlq"][rs]
                    k2T = projT["wlk2"][rs]
                else:
                    # matmul rhs / DVE sources must share the base
                    # partition of their base-0 partners
                    lqT = wp.tile([64, S], f32, tag="s0l", name="s0l")
                    nc.scalar.copy(lqT, projT["wlq"][rs])
                    k2T = wp.tile([64, S], f32, tag="s1l", name="s1l")
                    nc.scalar.copy(k2T, projT["wlk2"][rs])

                # lk1 natural [s, d] via transposes
                lk1nat = wp.tile([128, SC, 64], f32, tag="lk1nat",
                                 name="lk1nat", bufs=2)
                base = lk1T.base_partition()
                idsl = slice(base, base + 64)
                for t in range(SC):
                    pt = ps_tr.tile([128, 128], f32, tag="tr")
                    nc.tensor.transpose(
                        pt[:, :64], lk1T[:, t * 128:(t + 1) * 128],
                        ident[idsl, idsl])
                    nc.any.tensor_copy(lk1nat[:, t], pt[:, :64])
                # M = lk1^T @ lk1 [64, 64] (symmetric), fp32
                mps = ps_mm.tile([128, 512], f32, tag="mm", name="mps")
                for t in range(SC):
                    nc.tensor.matmul(mps[:64, :64], lhsT=lk1nat[:, t],
                                     rhs=lk1nat[:, t],
                                     start=(t == 0), stop=(t == SC - 1))
                m_sb = wp.tile([64, 64], f32, tag="m_sb", name="m_sb",
                               bufs=2)
                nc.any.tensor_copy(m_sb, mps[:64, :64])
                # qaug = (lq @ M)^T = M @ lq^T (M symmetric), fp32
                qaug = wp.tile([64, S], f32, tag="qaug", name="qaug",
                               bufs=2)
                for ic in range(4):
                    mm = ps_mm.tile([128, 512], f32, tag="mm", name="mm")
                    nc.tensor.matmul(mm[:64], lhsT=m_sb,
                                     rhs=lqT[:, ic * 512:(ic + 1) * 512],
                                     start=True, stop=True)
                    nc.any.tensor_copy(qaug[:, ic * 512:(ic + 1) * 512],
                                       mm[:64])
                # split pairs for pass 2:
                #   qhi65: rows 0:64 = qhi, row 64 = -max (pass 1)
                #   qcross: rows 0:64 = qhi, rows 64:128 = qlo
                #   k2hi65: rows 0:64 = k2hi, row 64 = ones
                #   k2cross: rows 0:64 = k2lo, rows 64:128 = k2hi
                qhi65 = wp.tile([65, S], f32r, tag="qhi65", name="qhi65",
                                bufs=2)
                qcross = wp.tile([128, S], f32r, tag="qcross",
                                 name="qcross", bufs=2)
                k2hi65 = wp.tile([65, S], f32r, tag="k2hi65",
                                 name="k2hi65", bufs=2)
                k2cross = wp.tile([128, S], f32r, tag="k2cross",
                                  name="k2cross", bufs=2)
                dekker_split(qaug, qhi65[:64], qcross[:64], qcross[64:128],
                             wp)
                dekker_split(k2T, k2hi65[:64], k2cross[64:128],
                             k2cross[:64], wp)
                # f32r memset is not a legal ISA value type; write the
                # ones row as row0 * 0 + 1 instead
                nc.vector.tensor_scalar(k2hi65[64:65, :], k2hi65[0:1, :],
                                        0.0, 1.0, op0=ALU.mult,
                                        op1=ALU.add)
                vaug = build_vaug_f32r(projT["wlv"][rs], wp)
                return dict(qhi65=qhi65, qcross=qcross, k2hi65=k2hi65,
                            k2cross=k2cross, vaug=vaug)

            def local_pass1(head, hs, wp):
                # hi-only scores, untransposed s[i, j]; row max via free-dim
                # reduce (B only needs ~+-70 accuracy in exponent units)
                qhi, k2hi = hs["qhi65"], hs["k2hi65"]
                maxneg = wp.tile([128, SC], f32, tag="maxneg", name="maxneg",
                                 bufs=2)
                for t in range(SC):
                    pmax = wp.tile([128, 4], f32, tag="pmax", name="pmax",
                                   bufs=2)
                    for j4 in range(4):
                        st = ps_mm.tile([128, 512], f32, tag="mm", name="st1")
                        nc.tensor.matmul(
                            st, lhsT=qhi[:64, t * 128:(t + 1) * 128],
                            rhs=k2hi[:64, j4 * 512:(j4 + 1) * 512],
                            start=True, stop=True)
                        nc.vector.tensor_reduce(pmax[:, j4:j4 + 1], st,
                                                axis=AX.X, op=ALU.max)
                    nc.vector.tensor_reduce(maxneg[:, t:t + 1], pmax,
                                            axis=AX.X, op=ALU.max,
                                            negate=True)
                mscr = dramp.tile([S], f32r, tag="mscr", name="mscr")
                nc.gpsimd.dma_start(
                    mscr.rearrange("(t p) -> p t", p=128), maxneg)
                nc.sync.dma_start(hs["qhi65"][64:65, :], mscr[None, :])

            def local_attention_ic(head, hs, ic):
                isl = slice(ic * 512, (ic + 1) * 512)
                ctx = ps_ctx.tile([65, 512], f32, tag="ctx", name="ctx")
                vaug = hs["vaug"]

                def ctx_group(es):
                    for jc, e in es:
                        nc.tensor.matmul(ctx, lhsT=vaug[:, jc], rhs=e,
                                         start=(jc == 0),
                                         stop=(jc == SC - 1))

                prev = None
                for jg in range(4):
                    es = []
                    for jj in range(4):
                        jc = jg * 4 + jj
                        jsl = slice(jc * 128, (jc + 1) * 128)
                        st = ps_mm.tile([128, 512], f32, tag="mm",
                                        name="st")
                        nc.tensor.matmul(st, lhsT=hs["k2hi65"][:, jsl],
                                         rhs=hs["qhi65"][:, isl],
                                         start=True, stop=False)
                        nc.tensor.matmul(st, lhsT=hs["k2cross"][:, jsl],
                                         rhs=hs["qcross"][:, isl],
                                         start=False, stop=True)
                        e = ep.tile([128, 512], f32r, tag="e", name="e",
                                    bufs=8)
                        nc.scalar.activation(e, st, AF.Exp, bias=0.0,
                                             scale=SCALE)
                        es.append((jc, e))
                    if prev is not None:
                        ctx_group(prev)
                    prev = es
                ctx_group(prev)
                out_path(ctx, head, ic)

            # ---------- phase A: projections + global heads ----------
            with (
                tc.tile_pool(name="pp_g", bufs=1) as pp_g,
                tc.tile_pool(name="hidT", bufs=1) as hp,
                tc.tile_pool(name="io", bufs=2) as iop,
                tc.tile_pool(name="lw", bufs=1) as lwp,
                tc.tile_pool(name="hlo", bufs=2) as hlop,
                tc.tile_pool(name="gv", bufs=1) as gvp,
            ):
                for n in ["wq", "wk", "wv"]:
                    projT[n] = pp_g.tile([128, S], f32r, name=f"projT_{n}")
                hhi_sb = hp.tile([128, HC, S], f32r, name="hhi_sb")
                hhi_r = hhi_d.rearrange("(c p) s -> p c s", p=128)
                # qkv weights first on the gpsimd queue so the first
                # projection matmuls start as soon as hhi chunk 0 lands
                wsb_g = {}
                for n in W_GLOBAL:
                    wsb_g[n] = iop.tile([128, HC, 128], f32r, tag="wg",
                                        name=f"w_{n}", bufs=2)
                    nc.gpsimd.dma_start(
                        wsb_g[n],
                        whi_d[n].rearrange("(c p) m -> p c m", p=128))
                for hc in range(HC):
                    eng = nc.sync if hc % 2 == 0 else nc.gpsimd
                    eng.dma_start(hhi_sb[:, hc], hhi_r[:, hc])
                # local weights (hi + lo) resident for the split projs
                wsb_l, wlo_l = {}, {}
                for n in W_LOCAL:
                    wsb_l[n] = lwp.tile([128, HC, 128], f32r,
                                        name=f"whi_{n}")
                    nc.gpsimd.dma_start(
                        wsb_l[n],
                        whi_d[n].rearrange("(c p) m -> p c m", p=128))
                    wlo_l[n] = lwp.tile([128, HC, 128], f32r,
                                        name=f"wlo_{n}")
                    nc.gpsimd.dma_start(
                        wlo_l[n],
                        wlo_d[n].rearrange("(c p) m -> p c m", p=128))

                # global projections: f32r hi-only, 2 psum accs per half
                def emit_gproj_half(n, half):
                    accs = [ps_mm.tile([128, 512], f32, tag="mm",
                                       name=f"acc{i}") for i in range(2)]
                    for hc in range(HC):
                        for i2 in range(2):
                            icg = half * 2 + i2
                            nc.tensor.matmul(
                                accs[i2], lhsT=wsb_g[n][:, hc],
                                rhs=hhi_sb[:, hc, icg * 512:(icg + 1) * 512],
                                start=(hc == 0), stop=(hc == HC - 1))
                    for i2 in range(2):
                        icg = half * 2 + i2
                        nc.vector.tensor_scalar_add(
                            projT[n][:, icg * 512:(icg + 1) * 512],
                            accs[i2], bias_sb[n])

                for n in W_GLOBAL:
                    for half in range(2):
                        emit_gproj_half(n, half)

                gvaug2 = {}
                for hh in range(2):
                    rs = slice(hh * 64, (hh + 1) * 64)
                    gvaug2[hh] = build_vaug2_fp8(projT["wv"][rs], gvp)

                # local projections: split pair, one 512-col ic at a time,
                # 3 held psum accs + streamed hlo slices
                def emit_lproj_ic(ic):
                    isl = slice(ic * 512, (ic + 1) * 512)
                    accs = {n: ps_mm.tile([128, 512], f32, tag="mm",
                                          name=f"lacc_{n}")
                            for n in W_LOCAL}
                    for hc in range(HC):
                        hlo_t = hlop.tile([128, 512], f32r, tag="hlo",
                                          name="hlo_t")
                        nc.sync.dma_start(hlo_t, hlo_r[:, hc, isl])
                        for n in W_LOCAL:
                            nc.tensor.matmul(
                                accs[n], lhsT=wsb_l[n][:, hc],
                                rhs=hhi_sb[:, hc, isl],
                                start=(hc == 0), stop=False)
                            nc.tensor.matmul(
                                accs[n], lhsT=wlo_l[n][:, hc],
                                rhs=hhi_sb[:, hc, isl],
                                start=False, stop=False)
                            nc.tensor.matmul(
                                accs[n], lhsT=wsb_l[n][:, hc],
                                rhs=hlo_t,
                                start=False, stop=(hc == HC - 1))
                    for n in W_LOCAL:
                        nc.vector.tensor_scalar_add(
                            projT[n][:, isl], accs[n], bias_sb[n])

                hlo_r = hlo_d.rearrange("(c p) s -> p c s", p=128)
                # interleave global attention units with local proj groups
                units = [(h, c) for h in range(2) for c in range(4)]
                for i, (hh, ic) in enumerate(units):
                    rs = slice(hh * 64, (hh + 1) * 64)
                    global_attention_ic(hh, projT["wk"][rs],
                                        projT["wq"][rs], gvaug2[hh], ic)
                    if i % 2 == 1:
                        emit_lproj_ic(i // 2)

            # ---------- phase B: local heads ----------
            with tc.tile_pool(name="wp_l", bufs=1) as wp_l:
                st2 = local_prep(2, wp_l)
                st3 = local_prep(3, wp_l)
                local_pass1(2, st2, wp_l)
                local_pass1(3, st3, wp_l)
                for ic in range(4):
                    local_attention_ic(2, st2, ic)
                for ic in range(4):
                    local_attention_ic(3, st3, ic)

    nc.compile()
    return nc


def _patch_ldw_opt():
    # walrus ships with the LDWEIGHTS optimizer disabled; fp32 matmuls
    # pay a bundled weight reload per matmul, so try enabling the
    # optimizer (verified against the reference output by the caller).
    from concourse import bass_utils
    if getattr(bass_utils, "_ldw_patched", False):
        return
    orig = bass_utils.bir_verify_and_optimise

    def patched(*a, **k):
        orig_run = bass_utils.run_command

        def run2(cmd, **kw):
            cmd = [c.replace("--enable-ldw-opt=false",
                             "--enable-ldw-opt=true") for c in cmd]
            return orig_run(cmd, **kw)

        bass_utils.run_command = run2
        try:
            return orig(*a, **k)
        finally:
            bass_utils.run_command = orig_run

    bass_utils.bir_verify_and_optimise = patched
    bass_utils._ldw_patched = True


def kernel(**inputs):
    from concourse import bass_utils

    if os.environ.get("LDW_OPT", "0") == "1":
        _patch_ldw_opt()

    global LAST_RESULTS
    if "nc" not in _CACHE:
        _CACHE["nc"] = _build()
    nc = _CACHE["nc"]

    inputs = dict(inputs)
    wfull = {n: np.asarray(inputs[n], np.float32) for n in
             ["wq", "wk", "wv", "wlq", "wlk1", "wlk2"]}
    wfull["wlv"] = (np.asarray(inputs["wlv1"], np.float32)
                    + np.asarray(inputs["wlv2"], np.float32))
    bfull = {n: np.asarray(inputs["b" + n[1:]], np.float32) for n in
             ["wq", "wk", "wv", "wlq", "wlk1", "wlk2"]}
    bfull["wlv"] = (np.asarray(inputs["blv1"], np.float32)
                    + np.asarray(inputs["blv2"], np.float32))
    hs = np.ascontiguousarray(np.asarray(inputs["hidden_states"], np.float32))
    am = np.ascontiguousarray(np.asarray(inputs["attention_mask"], np.float32))

    in_maps = []
    for c in range(N_CORES):
        b, g = c // 4, c % 4
        csl = slice(128 * g, 128 * (g + 1))
        hsT = np.ascontiguousarray(hs[b].T)
        hhi = _trunc10(hsT)
        m = {"hhi": hhi, "hlo": np.ascontiguousarray(hsT - hhi),
             "mask": am[b, 0, 0]}
        for n in W_NAMES:
            w = np.ascontiguousarray(wfull[n][:, csl])
            whi = _trunc10(w)
            m["hi_" + n] = whi
            if n in W_LOCAL:
                m["lo_" + n] = np.ascontiguousarray(w - whi)
            m["b_" + n] = np.ascontiguousarray(bfull[n][csl])
        in_maps.append(m)

    res = bass_utils.run_bass_kernel_spmd(
        nc, in_maps, list(range(N_CORES)),
        tmpdir=os.environ.get("BASS_TMPDIR"))
    LAST_RESULTS = res

    out = np.zeros((B, S, HID), np.float32)
    for c in range(N_CORES):
        b, g = c // 4, c % 4
        o = res.results[c]["out"]
        out[b, :, 128 * g:128 * (g + 1)] = o[:, :128]
        out[b, :, 512 + 128 * g:512 + 128 * (g + 1)] = o[:, 128:]
    return out


# revision 18
# speedup vs baseline: 1.4316x; 1.1235x over previous
"""Trainium2 Bass kernel for nn_MixedAttention.

Full inputs in, full output out. Sharding: 8 cores = 2 (batch) x 4 (head
pairs). Each core computes 2 global + 2 local heads for one batch element.

Key algebraic rewrite for the local branch:
    lscores = (lq@lk1^T)@(lk1@lk2^T) = lq @ (lk1^T@lk1) @ lk2^T
with M = lk1^T@lk1 a [64,64] matrix -- turns a 2048^3 matmul chain into
two small matmuls plus one S x S matmul (30x less PE work).

Precision/dtype strategy (PE cost: fp32=4 cyc/row, f32r/bf16/fp16=1, fp8
DoubleRow=0.5). The local score chain needs ~1e-6 relative accuracy
(scores reach |s|~5000 in exponent units and softmax is near-one-hot, so
score error d flips argmaxes on ~P(top2 gap < d) rows). Instead of fp32
matmuls we use exact split pairs: x = hi + lo with hi = fp16(x) (10-bit
mantissa, passes through the PE's ~13-bit f32r rounding unchanged) and
lo = fp16(x - hi) (pair error ~2^-21 relative; fp16 subnormal flushes
only affect terms whose absolute contribution is ~1e-7 of the score).
Then
  x@w = hi@whi + hi@wlo + lo@whi (+ dropped lo@wlo ~ 2^-20)
runs at 3 cyc/row (projections) and the pass-2 score matmul runs at
2 cyc/row by stacking the cross terms into one K=128 matmul:
  st = [k2hi;ones]^T @ [qhi;-max]  +  [k2lo;k2hi]^T @ [qhi;qlo].
Everything not exp-amplified runs plain fp16 (global q/k/v, local v,
ctx matmuls, e tiles) or fp8e4 DoubleRow (global ctx: probs are
near-uniform, N_eff~1600, so fp8 quantization of e and v averages out).
fp16 over f32r everywhere: same 1 cyc/row, but half the DMA/SBUF,
FWL-eligible weight loads, and 2x DVE/ACT throughput.
CPU-simulated end-to-end rel err of this scheme: ~1.2e-3 (gate 2e-2).

Layout: scores are computed transposed st[j, i] = K_eff @ Q_eff^T so the
context matmul needs no transposed probs (lhsT = v_nat, rhs = e). v gets
an extra ones column so the softmax denominator falls out of the context
matmul for free. Global heads skip max subtraction entirely (mask folded
into the Exp bias); local heads get a row max from a hi-only fp16 pass
in the untransposed orientation (free-dim reduce_max, split across the
vector and gpsimd engines), and the -max correction rides the K=65
contraction row of pass 2.
"""

import math
import os
import sys

import numpy as np

sys.path.insert(0, "/opt/trn_rl_repo")

B, S, HID, HEAD = 2, 2048, 1024, 64
SC = S // 128  # 16 s-chunks of 128
HC = HID // 128  # 8 hidden chunks
N_CORES = 8
SCALE = 1.0 / math.sqrt(HEAD)

W_GLOBAL = ["wq", "wk", "wv", "wlv"]  # fp16 hi-only projections
W_LOCAL = ["wlq", "wlk1", "wlk2"]  # split-pair projections (score chain)
W_NAMES = W_GLOBAL + W_LOCAL

_CACHE = {}
LAST_RESULTS = None  # stash of BassKernelResults for test.py profiling


def _build():
    import concourse.mybir as mybir
    import concourse.tile as tile
    from concourse import bacc
    from concourse.masks import make_identity

    f32 = mybir.dt.float32
    f16 = mybir.dt.float16
    f8 = mybir.dt.float8e4
    AF = mybir.ActivationFunctionType
    ALU = mybir.AluOpType
    AX = mybir.AxisListType
    DR = mybir.MatmulPerfMode.DoubleRow

    nc = bacc.Bacc("TRN2", target_bir_lowering=False, debug=False,
                   enable_asserts=False)

    hhi_d = nc.dram_tensor("hhi", (HID, S), f16, kind="ExternalInput").ap()
    hlo_d = nc.dram_tensor("hlo", (HID, S), f16, kind="ExternalInput").ap()
    mask_d = nc.dram_tensor("mask", (S,), f32, kind="ExternalInput").ap()
    whi_d = {n: nc.dram_tensor("hi_" + n, (HID, 128), f16,
                               kind="ExternalInput").ap() for n in W_NAMES}
    wlo_d = {n: nc.dram_tensor("lo_" + n, (HID, 128), f16,
                               kind="ExternalInput").ap() for n in W_LOCAL}
    b_d = {n: nc.dram_tensor("b_" + n, (128,), f32,
                             kind="ExternalInput").ap() for n in W_NAMES}
    out_d = nc.dram_tensor("out", (S, 256), f32, kind="ExternalOutput").ap()

    with tile.TileContext(nc) as tc:
        with (
            tc.tile_pool(name="const", bufs=1) as constp,
            tc.tile_pool(name="persist", bufs=1) as pp,
            tc.tile_pool(name="wp_g", bufs=1) as wp_g,
            tc.tile_pool(name="epool", bufs=4) as ep,
            tc.tile_pool(name="opool", bufs=1) as op_,
            tc.tile_pool(name="ps_tr", bufs=2, space="PSUM") as ps_tr,
            tc.tile_pool(name="ps_mm", bufs=4, space="PSUM") as ps_mm,
            tc.tile_pool(name="ps_ctx", bufs=2, space="PSUM") as ps_ctx,
            tc.tile_pool(name="dramp", bufs=2, space="DRAM") as dramp,
        ):
            ident = constp.tile([128, 128], f32, name="ident")
            make_identity(nc, ident)
            identh = constp.tile([128, 128], f16, name="identh")
            nc.vector.tensor_copy(identh, ident)
            ones_sb = constp.tile([128, SC], f32, name="ones_sb")
            nc.vector.memset(ones_sb, 1.0)
            mask_sb = constp.tile([128, SC], f32, name="mask_sb")
            nc.gpsimd.dma_start(mask_sb,
                                mask_d.rearrange("(c p) -> p c", p=128))
            bias_sb = {}
            for n in W_NAMES:
                t = constp.tile([128, 1], f32, name=f"b_{n}")
                nc.gpsimd.dma_start(t, b_d[n][:, None])
                bias_sb[n] = t

            # persistent (phase A + B) transposed projections
            projT = {n: pp.tile([128, S], f32, name=f"projT_{n}")
                     for n in W_LOCAL}
            projT["wlv"] = pp.tile([128, S], f16, name="projT_wlv")

            out_sb = op_.tile([128, SC, 256], f32, name="out_sb")

            # ---------- shared output path ----------

            def out_path(ctx, head, ic):
                csl = slice(head * 64, (head + 1) * 64)
                ctx_sbc = wp_g.tile([65, 512], f16, tag="ctx_sbc",
                                    name="ctx_sbc", bufs=2)
                nc.any.tensor_copy(ctx_sbc, ctx)
                for tt in range(4):
                    t = ic * 4 + tt
                    pt = ps_tr.tile([128, 128], f16, tag="tr")
                    nc.tensor.transpose(
                        pt[:, :65], ctx_sbc[:, tt * 128:(tt + 1) * 128],
                        identh[:65, :65])
                    rec = wp_g.tile([128, 1], f32, tag="rec", name="rec",
                                    bufs=2)
                    nc.vector.reciprocal(rec, pt[:, 64:65])
                    nc.vector.tensor_scalar_mul(
                        out_sb[:, t, csl], pt[:, :64], rec)
                nc.sync.dma_start(
                    out_d.rearrange("(t p) c -> p t c", p=128)[
                        :, ic * 4:(ic + 1) * 4, csl],
                    out_sb[:, ic * 4:(ic + 1) * 4, csl])

            # ---------- global branch ----------

            def global_attention_ic(hh, kT, qT, vaug2, ic):
                isl = slice(ic * 512, (ic + 1) * 512)
                ctx = ps_ctx.tile([65, 512], f32, tag="ctx", name="ctx")

                def ctx_pairs(pairs):
                    for g, e2 in pairs:
                        nc.tensor.matmul(ctx, lhsT=vaug2[:, g, :, :65],
                                         rhs=e2, start=(g == 0),
                                         stop=(g == 7), perf_mode=DR)

                prev = None
                for jg in range(4):
                    pairs = []
                    for jj in range(2):
                        e2 = ep.tile([128, 2, 512], f8, tag="e2", name="e2")
                        for i2 in range(2):
                            jc = jg * 4 + jj * 2 + i2
                            jsl = slice(jc * 128, (jc + 1) * 128)
                            st = ps_mm.tile([128, 512], f32, tag="mm",
                                            name="st")
                            nc.tensor.matmul(st, lhsT=kT[:, jsl],
                                             rhs=qT[:, isl],
                                             start=True, stop=True)
                            nc.scalar.activation(
                                e2[:, i2], st, AF.Exp,
                                bias=mask_sb[:, jc:jc + 1], scale=SCALE)
                        pairs.append((jg * 2 + jj, e2))
                    if prev is not None:
                        ctx_pairs(prev)
                    prev = pairs
                ctx_pairs(prev)
                out_path(ctx, hh, ic)

            def build_vaug2_fp8(vT, gvp):
                # v natural [s, d] + ones col, fp8, paired for DoubleRow:
                # [128, 8, 2, 80] where [:, g, i, d] = v[(2g+i)*128 + p, d]
                base = vT.base_partition()
                idsl = slice(base, base + 64)
                vaug2 = gvp.tile([128, 8, 2, 80], f8, tag="vaug2",
                                 name="vaug2", bufs=2)
                nc.vector.tensor_copy(
                    vaug2[:, :, :, 64],
                    ones_sb.rearrange("p (a b) -> p a b", a=8))
                for t in range(SC):
                    pt = ps_tr.tile([128, 128], f16, tag="tr")
                    nc.tensor.transpose(
                        pt[:, :64], vT[:, t * 128:(t + 1) * 128],
                        identh[idsl, idsl])
                    nc.any.tensor_copy(vaug2[:, t // 2, t % 2, :64],
                                       pt[:, :64])
                return vaug2

            # ---------- local branch ----------

            def build_vaug_f16(vT, wp):
                base = vT.base_partition()
                idsl = slice(base, base + 64)
                vaug = wp.tile([128, SC, 65], f16, tag="vaug",
                               name="vaug", bufs=2)
                nc.vector.tensor_copy(vaug[:, :, 64], ones_sb)
                for t in range(SC):
                    pt = ps_tr.tile([128, 128], f16, tag="tr")
                    nc.tensor.transpose(
                        pt[:, :64], vT[:, t * 128:(t + 1) * 128],
                        identh[idsl, idsl])
                    nc.any.tensor_copy(vaug[:, t, :64], pt[:, :64])
                return vaug

            def fp16_split(src, hi_dst, hi_dst2, lo_dst):
                # exact split: hi = fp16(src) has <=10 explicit mantissa
                # bits (passes through PE f32r rounding unchanged),
                # lo = fp16(src - hi) (~2^-21 pair error)
                nc.vector.tensor_copy(hi_dst, src)
                nc.vector.tensor_sub(lo_dst, src, hi_dst)
                nc.scalar.copy(hi_dst2, hi_dst)

            def local_prep(head, wp):
                hh = head % 2
                rs = slice(hh * 64, (hh + 1) * 64)
                lk1T = projT["wlk1"][rs]
                if hh == 0:
                    lqT = projT["wlq"][rs]
                    k2T = projT["wlk2"][rs]
                else:
                    # matmul rhs / DVE sources must share the base
                    # partition of their base-0 partners
                    lqT = wp.tile([64, S], f32, tag="s0l", name="s0l")
                    nc.scalar.copy(lqT, projT["wlq"][rs])
                    k2T = wp.tile([64, S], f32, tag="s1l", name="s1l")
                    nc.scalar.copy(k2T, projT["wlk2"][rs])

                # lk1 natural [s, d] via transposes (f32: score chain)
                lk1nat = wp.tile([128, SC, 64], f32, tag="lk1nat",
                                 name="lk1nat", bufs=2)
                base = lk1T.base_partition()
                idsl = slice(base, base + 64)
                for t in range(SC):
                    pt = ps_tr.tile([128, 128], f32, tag="tr")
                    nc.tensor.transpose(
                        pt[:, :64], lk1T[:, t * 128:(t + 1) * 128],
                        ident[idsl, idsl])
                    nc.any.tensor_copy(lk1nat[:, t], pt[:, :64])
                # M = lk1^T @ lk1 [64, 64] (symmetric), fp32
                mps = ps_mm.tile([128, 512], f32, tag="mm", name="mps")
                for t in range(SC):
                    nc.tensor.matmul(mps[:64, :64], lhsT=lk1nat[:, t],
                                     rhs=lk1nat[:, t],
                                     start=(t == 0), stop=(t == SC - 1))
                m_sb = wp.tile([64, 64], f32, tag="m_sb", name="m_sb",
                               bufs=2)
                nc.any.tensor_copy(m_sb, mps[:64, :64])
                # qaug = (lq @ M)^T = M @ lq^T (M symmetric), fp32
                qaug = wp.tile([64, S], f32, tag="qaug", name="qaug",
                               bufs=2)
                for ic in range(4):
                    mm = ps_mm.tile([128, 512], f32, tag="mm", name="mm")
                    nc.tensor.matmul(mm[:64], lhsT=m_sb,
                                     rhs=lqT[:, ic * 512:(ic + 1) * 512],
                                     start=True, stop=True)
                    nc.any.tensor_copy(qaug[:, ic * 512:(ic + 1) * 512],
                                       mm[:64])
                # split pairs for pass 2:
                #   qhi65: rows 0:64 = qhi, row 64 = -max (pass 1)
                #   qcross: rows 0:64 = qhi, rows 64:128 = qlo
                #   k2hi65: rows 0:64 = k2hi, row 64 = ones
                #   k2cross: rows 0:64 = k2lo, rows 64:128 = k2hi
                qhi65 = wp.tile([65, S], f16, tag="qhi65", name="qhi65",
                                bufs=2)
                qcross = wp.tile([128, S], f16, tag="qcross",
                                 name="qcross", bufs=2)
                k2hi65 = wp.tile([65, S], f16, tag="k2hi65",
                                 name="k2hi65", bufs=2)
                k2cross = wp.tile([128, S], f16, tag="k2cross",
                                  name="k2cross", bufs=2)
                fp16_split(qaug, qhi65[:64], qcross[:64], qcross[64:128])
                fp16_split(k2T, k2hi65[:64], k2cross[64:128],
                           k2cross[:64])
                # f16 memset is not a legal ISA value type; write the
                # ones row as row0 * 0 + 1 instead
                nc.vector.tensor_scalar(k2hi65[64:65, :], k2hi65[0:1, :],
                                        0.0, 1.0, op0=ALU.mult,
                                        op1=ALU.add)
                vaug = build_vaug_f16(projT["wlv"][rs], wp)
                return dict(qhi65=qhi65, qcross=qcross, k2hi65=k2hi65,
                            k2cross=k2cross, vaug=vaug)

            def local_pass1(head, hs, wp):
                # hi-only scores, untransposed s[i, j]; row max via free-dim
                # reduce split across vector+gpsimd (B only needs ~+-70
                # accuracy in exponent units). raw |s|max ~ 4e4 < fp16 max.
                qhi, k2hi = hs["qhi65"], hs["k2hi65"]
                maxneg = wp.tile([128, SC], f16, tag="maxneg", name="maxneg",
                                 bufs=2)
                for t in range(SC):
                    pmax = wp.tile([128, 4], f32, tag="pmax", name="pmax",
                                   bufs=2)
                    for j4 in range(4):
                        st = ps_mm.tile([128, 512], f32, tag="mm", name="st1")
                        nc.tensor.matmul(
                            st, lhsT=qhi[:64, t * 128:(t + 1) * 128],
                            rhs=k2hi[:64, j4 * 512:(j4 + 1) * 512],
                            start=True, stop=True)
                        nc.vector.tensor_reduce(pmax[:, j4:j4 + 1], st,
                                                axis=AX.X, op=ALU.max)
                    nc.vector.tensor_reduce(maxneg[:, t:t + 1], pmax,
                                            axis=AX.X, op=ALU.max,
                                            negate=True)
                mscr = dramp.tile([S], f16, tag="mscr", name="mscr")
                nc.sync.dma_start(
                    mscr.rearrange("(t p) -> p t", p=128), maxneg)
                nc.sync.dma_start(hs["qhi65"][64:65, :], mscr[None, :])

            def local_attention_ic(head, hs, ic):
                isl = slice(ic * 512, (ic + 1) * 512)
                ctx = ps_ctx.tile([65, 512], f32, tag="ctx", name="ctx")
                vaug = hs["vaug"]

                def ctx_group(es):
                    for jc, e in es:
                        nc.tensor.matmul(ctx, lhsT=vaug[:, jc], rhs=e,
                                         start=(jc == 0),
                                         stop=(jc == SC - 1))

                prev = None
                for jg in range(4):
                    es = []
                    for jj in range(4):
                        jc = jg * 4 + jj
                        jsl = slice(jc * 128, (jc + 1) * 128)
                        st = ps_mm.tile([128, 512], f32, tag="mm",
                                        name="st")
                        nc.tensor.matmul(st, lhsT=hs["k2hi65"][:, jsl],
                                         rhs=hs["qhi65"][:, isl],
                                         start=True, stop=False)
                        nc.tensor.matmul(st, lhsT=hs["k2cross"][:, jsl],
                                         rhs=hs["qcross"][:, isl],
                                         start=False, stop=True)
                        e = ep.tile([128, 512], f16, tag="e", name="e",
                                    bufs=8)
                        nc.scalar.activation(e, st, AF.Exp, bias=0.0,
                                             scale=SCALE)
                        es.append((jc, e))
                    if prev is not None:
                        ctx_group(prev)
                    prev = es
                ctx_group(prev)
                out_path(ctx, head, ic)

            # ---------- phase A: projections + global heads ----------
            with (
                tc.tile_pool(name="pp_g", bufs=1) as pp_g,
                tc.tile_pool(name="hidT", bufs=1) as hp,
                tc.tile_pool(name="io", bufs=2) as iop,
                tc.tile_pool(name="lw", bufs=1) as lwp,
                tc.tile_pool(name="gv", bufs=1) as gvp,
            ):
                for n in ["wq", "wk", "wv"]:
                    projT[n] = pp_g.tile([128, S], f16, name=f"projT_{n}")
                hhi_sb = hp.tile([128, HC, S], f16, name="hhi_sb")
                hlo_sb = hp.tile([128, HC, S], f16, name="hlo_sb")
                hhi_r = hhi_d.rearrange("(c p) s -> p c s", p=128)
                hlo_r = hlo_d.rearrange("(c p) s -> p c s", p=128)
                # spread input DMA across 4 queues; weights on gpsimd.
                # wq weights + hhi chunk 0 land first so the PE starts
                # within a few us.
                dqs = [nc.sync, nc.scalar]
                wsb_g = {}
                for n in W_GLOBAL:
                    wsb_g[n] = iop.tile([128, HC, 128], f16, tag="wg",
                                        name=f"w_{n}", bufs=2)
                    nc.gpsimd.dma_start(
                        wsb_g[n],
                        whi_d[n].rearrange("(c p) m -> p c m", p=128))
                for hc in range(HC):
                    dqs[hc % 2].dma_start(hhi_sb[:, hc], hhi_r[:, hc])
                wsb_l, wlo_l = {}, {}
                for n in W_LOCAL:
                    wsb_l[n] = lwp.tile([128, HC, 128], f16,
                                        name=f"whi_{n}")
                    nc.gpsimd.dma_start(
                        wsb_l[n],
                        whi_d[n].rearrange("(c p) m -> p c m", p=128))
                    wlo_l[n] = lwp.tile([128, HC, 128], f16,
                                        name=f"wlo_{n}")
                    nc.gpsimd.dma_start(
                        wlo_l[n],
                        wlo_d[n].rearrange("(c p) m -> p c m", p=128))
                for hc in range(HC):
                    dqs[hc % 2].dma_start(hlo_sb[:, hc], hlo_r[:, hc])

                # global projections: fp16 hi-only, 2 psum accs per half
                def emit_gproj_half(n, half):
                    accs = [ps_mm.tile([128, 512], f32, tag="mm",
                                       name=f"acc{i}") for i in range(2)]
                    for hc in range(HC):
                        for i2 in range(2):
                            icg = half * 2 + i2
                            nc.tensor.matmul(
                                accs[i2], lhsT=wsb_g[n][:, hc],
                                rhs=hhi_sb[:, hc, icg * 512:(icg + 1) * 512],
                                start=(hc == 0), stop=(hc == HC - 1))
                    for i2 in range(2):
                        icg = half * 2 + i2
                        nc.vector.tensor_scalar_add(
                            projT[n][:, icg * 512:(icg + 1) * 512],
                            accs[i2], bias_sb[n])

                for n in W_GLOBAL:
                    for half in range(2):
                        emit_gproj_half(n, half)

                gvaug2 = {}
                for hh in range(2):
                    rs = slice(hh * 64, (hh + 1) * 64)
                    gvaug2[hh] = build_vaug2_fp8(projT["wv"][rs], gvp)

                # local projections: split pair (3 matmuls per chunk,
                # ordered so the two whi uses are adjacent for weight
                # reuse), one 512-col ic group at a time
                def emit_lproj_ic(ic):
                    isl = slice(ic * 512, (ic + 1) * 512)
                    accs = {n: ps_mm.tile([128, 512], f32, tag="mm",
                                          name=f"lacc_{n}")
                            for n in W_LOCAL}
                    for hc in range(HC):
                        for n in W_LOCAL:
                            nc.tensor.matmul(
                                accs[n], lhsT=wsb_l[n][:, hc],
                                rhs=hhi_sb[:, hc, isl],
                                start=(hc == 0), stop=False)
                            nc.tensor.matmul(
                                accs[n], lhsT=wsb_l[n][:, hc],
                                rhs=hlo_sb[:, hc, isl],
                                start=False, stop=False)
                            nc.tensor.matmul(
                                accs[n], lhsT=wlo_l[n][:, hc],
                                rhs=hhi_sb[:, hc, isl],
                                start=False, stop=(hc == HC - 1))
                    for n in W_LOCAL:
                        nc.vector.tensor_scalar_add(
                            projT[n][:, isl], accs[n], bias_sb[n])

                # interleave global attention units with local proj groups
                units = [(h, c) for h in range(2) for c in range(4)]
                for i, (hh, ic) in enumerate(units):
                    rs = slice(hh * 64, (hh + 1) * 64)
                    global_attention_ic(hh, projT["wk"][rs],
                                        projT["wq"][rs], gvaug2[hh], ic)
                    if i % 2 == 1:
                        emit_lproj_ic(i // 2)

            # ---------- phase B: local heads ----------
            with tc.tile_pool(name="wp_l", bufs=1) as wp_l:
                st2 = local_prep(2, wp_l)
                local_pass1(2, st2, wp_l)
                st3 = local_prep(3, wp_l)
                local_pass1(3, st3, wp_l)
                for ic in range(4):
                    local_attention_ic(2, st2, ic)
                for ic in range(4):
                    local_attention_ic(3, st3, ic)

    nc.compile()
    return nc


def _patch_ldw_opt():
    # walrus ships with the LDWEIGHTS optimizer disabled; fp32 matmuls
    # pay a bundled weight reload per matmul, so try enabling the
    # optimizer (verified against the reference output by the caller).
    from concourse import bass_utils
    if getattr(bass_utils, "_ldw_patched", False):
        return
    orig = bass_utils.bir_verify_and_optimise

    def patched(*a, **k):
        orig_run = bass_utils.run_command

        def run2(cmd, **kw):
            cmd = [c.replace("--enable-ldw-opt=false",
                             "--enable-ldw-opt=true") for c in cmd]
            return orig_run(cmd, **kw)

        bass_utils.run_command = run2
        try:
            return orig(*a, **k)
        finally:
            bass_utils.run_command = orig_run

    bass_utils.bir_verify_and_optimise = patched
    bass_utils._ldw_patched = True


def kernel(**inputs):
    from concourse import bass_utils

    if os.environ.get("LDW_OPT", "0") == "1":
        _patch_ldw_opt()

    global LAST_RESULTS
    if "nc" not in _CACHE:
        _CACHE["nc"] = _build()
    nc = _CACHE["nc"]

    inputs = dict(inputs)
    wfull = {n: np.asarray(inputs[n], np.float32) for n in
             ["wq", "wk", "wv", "wlq", "wlk1", "wlk2"]}
    wfull["wlv"] = (np.asarray(inputs["wlv1"], np.float32)
                    + np.asarray(inputs["wlv2"], np.float32))
    bfull = {n: np.asarray(inputs["b" + n[1:]], np.float32) for n in
             ["wq", "wk", "wv", "wlq", "wlk1", "wlk2"]}
    bfull["wlv"] = (np.asarray(inputs["blv1"], np.float32)
                    + np.asarray(inputs["blv2"], np.float32))
    hs = np.ascontiguousarray(np.asarray(inputs["hidden_states"], np.float32))
    am = np.ascontiguousarray(np.asarray(inputs["attention_mask"], np.float32))

    in_maps = []
    for c in range(N_CORES):
        b, g = c // 4, c % 4
        csl = slice(128 * g, 128 * (g + 1))
        hsT = np.ascontiguousarray(hs[b].T)
        hhi = hsT.astype(np.float16)
        m = {"hhi": hhi,
             "hlo": (hsT - hhi.astype(np.float32)).astype(np.float16),
             "mask": am[b, 0, 0]}
        for n in W_NAMES:
            w = np.ascontiguousarray(wfull[n][:, csl])
            whi = w.astype(np.float16)
            m["hi_" + n] = whi
            if n in W_LOCAL:
                m["lo_" + n] = (w - whi.astype(np.float32)).astype(
                    np.float16)
            m["b_" + n] = np.ascontiguousarray(bfull[n][csl])
        in_maps.append(m)

    res = bass_utils.run_bass_kernel_spmd(
        nc, in_maps, list(range(N_CORES)),
        tmpdir=os.environ.get("BASS_TMPDIR"))
    LAST_RESULTS = res

    out = np.zeros((B, S, HID), np.float32)
    for c in range(N_CORES):
        b, g = c // 4, c % 4
        o = res.results[c]["out"]
        out[b, :, 128 * g:128 * (g + 1)] = o[:, :128]
        out[b, :, 512 + 128 * g:512 + 128 * (g + 1)] = o[:, 128:]
    return out
